# revision 28
# baseline (speedup 1.0000x reference)
"""Trainium2 Bass kernel for GQA causal attention (B=2, T=2048, H=16, KV=4, D=128).

Sharding: 8 cores = (batch b in {0,1}) x (kv-group g in {0..3}).
Each core computes 4 Q heads + 1 KV head for one batch:
  Q/K/V projections (column-parallel weights), RoPE, causal attention,
  row-parallel Wo partials, ReduceScatter within the 4-core batch group.
Each core returns its 512 summed output rows; the host reassembles.
"""

import math

import numpy as np

import concourse.mybir as mybir
import concourse.tile as tile
from concourse import bacc
from concourse.bass_utils import run_bass_kernel_spmd
from concourse.masks import make_identity

F32 = mybir.dt.float32
BF16 = mybir.dt.bfloat16
EXP = mybir.ActivationFunctionType.Exp
MULT = mybir.AluOpType.mult

B, T, C = 2, 2048, 2048
H, KH, D = 16, 4, 128
R = H // KH  # q heads per kv group (4)
N_CORES = 8
TI = T // 128  # 16 token blocks
EO = C // 128  # 16 embedding chunks
SCALE = 1.0 / math.sqrt(D)

NG = 2  # pipeline groups over tokens
GT = T // NG  # tokens per group (1024)
GB = GT // 128  # 128-blocks per group (8)
NROW = GT // KH  # rows each core owns per group (256)

_CACHE = {}


def _build_program():
    nc = bacc.Bacc(
        "TRN2", target_bir_lowering=False, debug=False, num_devices=N_CORES
    )

    x_d = nc.dram_tensor("x", [T, C], F32, kind="ExternalInput").ap()
    cos_d = nc.dram_tensor("cos", [T, D], F32, kind="ExternalInput").ap()
    sin_d = nc.dram_tensor("sin", [T, D], F32, kind="ExternalInput").ap()
    wq_d = nc.dram_tensor("wq", [C, R * D], F32, kind="ExternalInput").ap()
    wk_d = nc.dram_tensor("wk", [C, D], F32, kind="ExternalInput").ap()
    wv_d = nc.dram_tensor("wv", [C, D], F32, kind="ExternalInput").ap()
    wo_d = nc.dram_tensor("wo", [R * D, C], F32, kind="ExternalInput").ap()
    out_d = nc.dram_tensor("out", [NG * NROW, C], F32, kind="ExternalOutput").ap()

    with tile.TileContext(nc) as tc:
        _kernel_body(tc, x_d, cos_d, sin_d, wq_d, wk_d, wv_d, wo_d, out_d)

    nc.compile()
    return nc


def _kernel_body(tc, x_d, cos_d, sin_d, wq_d, wk_d, wv_d, wo_d, out_d):
    nc = tc.nc

    consts = tc.alloc_tile_pool(name="consts", bufs=1)
    projout = tc.alloc_tile_pool(name="projout", bufs=1)
    wo_pool = tc.alloc_tile_pool(name="wo", bufs=1)
    dram = tc.alloc_tile_pool(name="dram", bufs=1, space="DRAM")

    # --- constants: causal mask (ST layout: keep tk<=tq), cos/sin, identity ---
    ut_mask = consts.tile([128, 128], BF16)
    nc.gpsimd.memset(ut_mask, 1.0)
    nc.gpsimd.affine_select(
        out=ut_mask,
        in_=ut_mask,
        compare_op=mybir.AluOpType.is_ge,
        fill=0.0,
        base=0,
        pattern=[[1, 128]],
        channel_multiplier=-1,
    )

    cos_sb = consts.tile([128, TI, D], F32)
    sin_sb = consts.tile([128, TI, D], F32)
    nc.gpsimd.dma_start(cos_sb, cos_d.rearrange("(to ti) d -> ti to d", ti=128))
    nc.gpsimd.dma_start(sin_sb, sin_d.rearrange("(to ti) d -> ti to d", ti=128))

    ident_f = consts.tile([128, 128], F32)
    make_identity(nc, ident_f)
    ident_b = consts.tile([128, 128], BF16)
    make_identity(nc, ident_b)

    # --- persistent proj outputs, split by token group for fine-grained deps ---
    qt_g = [projout.tile([128, R, GT], BF16, name=f"qt{g}") for g in range(NG)]  # [d, h, tok]
    kt_g = [projout.tile([128, GT], BF16, name=f"kt{g}") for g in range(NG)]  # [d, tok]
    v_g = [projout.tile([128, GB, 132], BF16, name=f"v{g}") for g in range(NG)]  # [tok, kb, d|1]
    yt_g = [projout.tile([128, R, GT], BF16, name=f"yt{g}") for g in range(NG)]  # [d, h, tok]
    for g in range(NG):
        nc.vector.memset(v_g[g][:, :, 128], 1.0)

    wo_bf = wo_pool.tile([128, R, C], BF16)  # [d, h, embd_out]

    # --- load weights (cast to bf16); x loads go first on the sync queue ---
    with tc.tile_pool(name="wts", bufs=1) as wts, tc.tile_pool(
        name="wstage", bufs=3
    ) as wstage:
        wq_bf = wts.tile([128, EO, R * D], BF16)
        wk_bf = wts.tile([128, EO, D], BF16)
        wv_bf = wts.tile([128, EO, D], BF16)
        for eo in range(EO):
            st_q = wstage.tile([128, R * D], F32, tag="wst")
            nc.gpsimd.dma_start(st_q, wq_d[eo * 128 : (eo + 1) * 128, :])
            nc.vector.tensor_copy(wq_bf[:, eo, :], st_q)
            st_k = wstage.tile([128, D], F32, tag="wst_kv")
            nc.gpsimd.dma_start(st_k, wk_d[eo * 128 : (eo + 1) * 128, :])
            nc.vector.tensor_copy(wk_bf[:, eo, :], st_k)
            st_v = wstage.tile([128, D], F32, tag="wst_kv")
            nc.gpsimd.dma_start(st_v, wv_d[eo * 128 : (eo + 1) * 128, :])
            nc.vector.tensor_copy(wv_bf[:, eo, :], st_v)
        for h in range(R):
            for no in range(4):
                st_o = wstage.tile([128, 512], F32, tag="wst")
                nc.gpsimd.dma_start(
                    st_o, wo_d[h * 128 : (h + 1) * 128, no * 512 : (no + 1) * 512]
                )
                nc.scalar.copy(wo_bf[:, h, no * 512 : (no + 1) * 512], st_o)

        # --- per token block: load x, PE-transpose, project Q/K/V, RoPE ---
        with tc.tile_pool(name="xstage", bufs=3) as xstage, tc.tile_pool(
            name="xtb", bufs=3
        ) as xtb_pool, tc.tile_pool(
            name="ppsum", bufs=2, space="PSUM"
        ) as ppsum, tc.tile_pool(
            name="kvpsum", bufs=1, space="PSUM"
        ) as kvpsum, tc.tile_pool(
            name="tpsum", bufs=2, space="PSUM"
        ) as tpsum, tc.tile_pool(
            name="qtpsum", bufs=2, space="PSUM"
        ) as qtpsum, tc.tile_pool(name="rope", bufs=2) as rope:
            for ti in range(TI):
                g = ti // GB
                tl = ti * 128 - g * GT  # local token offset within group
                qt_bf, kt_bf, v_bf = qt_g[g], kt_g[g], v_g[g]

                xf = xstage.tile([128, C], F32, tag="xf")
                nc.sync.dma_start(xf, x_d[ti * 128 : (ti + 1) * 128, :])
                xt_blk = xtb_pool.tile([128, EO, 128], BF16, tag="xt")
                for e4 in range(4):
                    tp = tpsum.tile([128, 4, 128], F32, tag="tp")
                    for k in range(4):
                        eo = e4 * 4 + k
                        nc.tensor.transpose(
                            tp[:, k, :], xf[:, eo * 128 : (eo + 1) * 128], ident_f
                        )
                    if e4 % 2 == 0:
                        nc.vector.tensor_copy(xt_blk[:, e4 * 4 : e4 * 4 + 4, :], tp)
                    else:
                        nc.scalar.copy(xt_blk[:, e4 * 4 : e4 * 4 + 4, :], tp)

                # Q: psum [tok, R*D]
                psq = ppsum.tile([128, R * D], F32, tag="psq")
                for eo in range(EO):
                    nc.tensor.matmul(
                        psq,
                        lhsT=xt_blk[:, eo, :],
                        rhs=wq_bf[:, eo, :],
                        start=(eo == 0),
                        stop=(eo == EO - 1),
                    )
                psq_v = psq[:, :].rearrange("p (h d) -> p h d", h=R)
                cos_bc = cos_sb[:, ti, None, :].to_broadcast((128, R, D))
                sin_bc = sin_sb[:, ti, None, :].to_broadcast((128, R, D))
                tc_t = rope.tile([128, R, D], F32, tag="ropeC")
                ts_t = rope.tile([128, R, D], F32, tag="ropeS")
                nc.vector.tensor_tensor(tc_t, psq_v, cos_bc, MULT)
                nc.vector.tensor_tensor(ts_t, psq_v, sin_bc, MULT)
                qb = rope.tile([128, R, D], BF16, tag="qb")
                nc.vector.tensor_sub(
                    qb[:, :, 0:64], tc_t[:, :, 0:64], ts_t[:, :, 64:128]
                )
                nc.vector.tensor_add(
                    qb[:, :, 64:128], tc_t[:, :, 64:128], ts_t[:, :, 0:64]
                )
                qtp = qtpsum.tile([128, R, 128], BF16, tag="qtp")
                for h in range(R):
                    nc.tensor.transpose(qtp[:, h, :], qb[:, h, :], ident_b)
                if ti % 2 == 0:
                    nc.vector.tensor_copy(qt_bf[:, :, tl : tl + 128], qtp)
                else:
                    nc.scalar.copy(qt_bf[:, :, tl : tl + 128], qtp)

                # K: psum [tok, D]
                psk = kvpsum.tile([128, D], F32, tag="pskv")
                for eo in range(EO):
                    nc.tensor.matmul(
                        psk,
                        lhsT=xt_blk[:, eo, :],
                        rhs=wk_bf[:, eo, :],
                        start=(eo == 0),
                        stop=(eo == EO - 1),
                    )
                tck = rope.tile([128, D], F32, tag="ropeCk")
                tsk = rope.tile([128, D], F32, tag="ropeSk")
                nc.vector.tensor_tensor(tck, psk, cos_sb[:, ti, :], MULT)
                nc.vector.tensor_tensor(tsk, psk, sin_sb[:, ti, :], MULT)
                kb_t = rope.tile([128, D], BF16, tag="kb")
                nc.vector.tensor_sub(kb_t[:, 0:64], tck[:, 0:64], tsk[:, 64:128])
                nc.vector.tensor_add(kb_t[:, 64:128], tck[:, 64:128], tsk[:, 0:64])
                tpk = qtpsum.tile([128, R, 128], BF16, tag="qtp")
                nc.tensor.transpose(tpk[:, 0, :], kb_t, ident_b)
                nc.scalar.copy(kt_bf[:, tl : tl + 128], tpk[:, 0, :])

                # V: psum [tok, D] -> v_bf[:, kb_local, 0:128]; col 128 = 1.0
                psv = kvpsum.tile([128, D], F32, tag="pskv")
                for eo in range(EO):
                    nc.tensor.matmul(
                        psv,
                        lhsT=xt_blk[:, eo, :],
                        rhs=wv_bf[:, eo, :],
                        start=(eo == 0),
                        stop=(eo == EO - 1),
                    )
                nc.scalar.copy(v_bf[:, ti - g * GB, 0:128], psv)

    # --- attention + Wo + ReduceScatter, pipelined over token groups ---
    groups = [[0, 1, 2, 3], [4, 5, 6, 7]]
    with tc.tile_pool(name="st", bufs=2) as stp, tc.tile_pool(
        name="apsum", bufs=2, space="PSUM"
    ) as apsum, tc.tile_pool(
        name="ytpsum", bufs=1, space="PSUM"
    ) as ytpsum, tc.tile_pool(
        name="wopsum", bufs=1, space="PSUM"
    ) as wopsum, tc.tile_pool(name="ypool", bufs=3) as ypool, tc.tile_pool(
        name="outp", bufs=2
    ) as outp:
        st_max = max(
            sum((g + 1) * GT - max(kb * 128, g * GT) for kb in range((g + 1) * GB))
            for g in range(NG)
        )
        for g in range(NG):
            lo, hi = g * GT, (g + 1) * GT
            offs = {}
            o = 0
            for kb in range((g + 1) * GB):
                offs[kb] = o
                o += hi - max(kb * 128, lo)

            for h in range(R):
                st_all = stp.tile([128, st_max], BF16, tag="st_all")
                for kb in range((g + 1) * GB):
                    s0 = max(kb * 128, lo)
                    w = hi - s0
                    ps = apsum.tile([128, 1024], F32, tag="strip")
                    for m0 in range(0, w, 512):
                        mw = min(512, w - m0)
                        nc.tensor.matmul(
                            ps[:, m0 : m0 + mw],
                            lhsT=kt_g[kb // GB][
                                :, kb * 128 - (kb // GB) * GT : (kb + 1) * 128 - (kb // GB) * GT
                            ],
                            rhs=qt_g[g][:, h, s0 - lo + m0 : s0 - lo + m0 + mw],
                            start=True,
                            stop=True,
                        )
                    nc.scalar.activation(
                        st_all[:, offs[kb] : offs[kb] + w],
                        ps[:, :w],
                        EXP,
                        scale=SCALE,
                    )
                    if kb * 128 >= lo:  # diagonal block lives in this group
                        nc.vector.tensor_mul(
                            st_all[:, offs[kb] : offs[kb] + 128],
                            st_all[:, offs[kb] : offs[kb] + 128],
                            ut_mask,
                        )
                # AV: per query block j in this group, accumulate over kb<=j
                for j in range(g * GB, (g + 1) * GB):
                    po = apsum.tile([128, 132], F32, tag="po")
                    for kb in range(j + 1):
                        s = offs[kb] + j * 128 - max(kb * 128, lo)
                        nc.tensor.matmul(
                            po[:, 0:129],
                            lhsT=st_all[:, s : s + 128],
                            rhs=v_g[kb // GB][:, kb - (kb // GB) * GB, 0:129],
                            start=(kb == 0),
                            stop=(kb == j),
                        )
                    rec = ypool.tile([128, 1], F32, tag="rec")
                    nc.vector.reciprocal(rec, po[:, 128:129])
                    yb = ypool.tile([128, 128], BF16, tag="yb")
                    nc.vector.tensor_scalar_mul(yb, po[:, 0:128], rec)
                    ytp = ytpsum.tile([128, 128], BF16, tag="ytp")
                    nc.tensor.transpose(ytp, yb, ident_b)
                    jl = j * 128 - lo
                    if j % 2 == 0:
                        nc.vector.tensor_copy(yt_g[g][:, h, jl : jl + 128], ytp)
                    else:
                        nc.scalar.copy(yt_g[g][:, h, jl : jl + 128], ytp)

            # Wo partial rows for this group (f32 copy + single bf16 cast)
            partial_g = dram.tile([GT, C], BF16, tag=f"partial{g}")
            for tb in range(GB):
                osf = outp.tile([128, C], F32, tag="osf")
                for no in range(4):
                    pw = wopsum.tile([128, 512], F32, tag="pw")
                    for hh in range(R):
                        nc.tensor.matmul(
                            pw,
                            lhsT=yt_g[g][:, hh, tb * 128 : (tb + 1) * 128],
                            rhs=wo_bf[:, hh, no * 512 : (no + 1) * 512],
                            start=(hh == 0),
                            stop=(hh == R - 1),
                        )
                    if no % 2 == 0:
                        nc.vector.tensor_copy(osf[:, no * 512 : (no + 1) * 512], pw)
                    else:
                        nc.scalar.copy(osf[:, no * 512 : (no + 1) * 512], pw)
                osb = outp.tile([128, C], BF16, tag="osb")
                nc.vector.tensor_copy(osb, osf)
                nc.sync.dma_start(partial_g[tb * 128 : (tb + 1) * 128, :], osb)

            rs_g = dram.tile([NROW, C], BF16, tag=f"rs{g}")
            nc.gpsimd.collective_compute(
                "ReduceScatter",
                mybir.AluOpType.add,
                replica_groups=groups,
                ins=[partial_g.opt()],
                outs=[rs_g.opt()],
            )
            # cast my 256-row share to f32 and store to local output rows
            for blk in range(NROW // 128):
                rsb = outp.tile([128, C], BF16, tag="rsb")
                nc.sync.dma_start(rsb, rs_g[blk * 128 : (blk + 1) * 128, :])
                osf2 = outp.tile([128, C], F32, tag="osf2")
                nc.vector.tensor_copy(osf2, rsb)
                nc.sync.dma_start(
                    out_d[g * NROW + blk * 128 : g * NROW + (blk + 1) * 128, :],
                    osf2,
                )

    for pool in (dram, wo_pool, projout, consts):
        pool.release()


def _shard_inputs(x, cos, sin, Wq, Wkv, Wo):
    in_maps = []
    for c in range(N_CORES):
        b, g = c // KH, c % KH
        in_maps.append(
            {
                "x": np.ascontiguousarray(x[b], dtype=np.float32),
                "cos": np.ascontiguousarray(cos, dtype=np.float32),
                "sin": np.ascontiguousarray(sin, dtype=np.float32),
                "wq": np.ascontiguousarray(
                    Wq[:, g * R * D : (g + 1) * R * D], dtype=np.float32
                ),
                "wk": np.ascontiguousarray(
                    Wkv[:, g * D : (g + 1) * D], dtype=np.float32
                ),
                "wv": np.ascontiguousarray(
                    Wkv[:, KH * D + g * D : KH * D + (g + 1) * D], dtype=np.float32
                ),
                "wo": np.ascontiguousarray(
                    Wo[g * R * D : (g + 1) * R * D, :], dtype=np.float32
                ),
            }
        )
    return in_maps


def get_program():
    if "nc" not in _CACHE:
        _CACHE["nc"] = _build_program()
    return _CACHE["nc"]


def run(x, cos, sin, Wq, Wkv, Wo, **spmd_kwargs):
    nc = get_program()
    in_maps = _shard_inputs(x, cos, sin, Wq, Wkv, Wo)
    res = run_bass_kernel_spmd(
        nc, in_maps, core_ids=list(range(N_CORES)), **spmd_kwargs
    )
    # core (b, r) holds rows [g*GT + r*NROW, +NROW) of batch b at local
    # offset [g*NROW, +NROW) for each token group g.
    out = np.empty((B, T, C), dtype=np.float32)
    for b in range(B):
        for r in range(KH):
            loc = res.results[b * KH + r]["out"]
            for g in range(NG):
                out[b, g * GT + r * NROW : g * GT + (r + 1) * NROW] = loc[
                    g * NROW : (g + 1) * NROW
                ]
    return out, res


def kernel(x, cos, sin, Wq, Wkv, Wo):
    out, _ = run(x, cos, sin, Wq, Wkv, Wo)
    return out


# revision 29
# speedup vs baseline: 1.0553x; 1.0553x over previous
"""Trainium2 Bass kernel for GQA causal attention (B=2, T=2048, H=16, KV=4, D=128).

Sharding: 8 cores = (batch b in {0,1}) x (kv-group g in {0..3}).
Each core computes 4 Q heads + 1 KV head for one batch:
  Q/K/V projections (column-parallel weights), RoPE, causal attention,
  row-parallel Wo partials, ReduceScatter within the 4-core batch group.
Each core returns its 512 summed output rows; the host reassembles.
"""

import math

import numpy as np

import concourse.mybir as mybir
import concourse.tile as tile
from concourse import bacc
from concourse.bass_utils import run_bass_kernel_spmd
from concourse.masks import make_identity

F32 = mybir.dt.float32
BF16 = mybir.dt.bfloat16
EXP = mybir.ActivationFunctionType.Exp
MULT = mybir.AluOpType.mult

B, T, C = 2, 2048, 2048
H, KH, D = 16, 4, 128
R = H // KH  # q heads per kv group (4)
N_CORES = 8
TI = T // 128  # 16 token blocks
EO = C // 128  # 16 embedding chunks
SCALE = 1.0 / math.sqrt(D)

NG = 2  # pipeline groups over tokens
GT = T // NG  # tokens per group (1024)
GB = GT // 128  # 128-blocks per group (8)
NROW = GT // KH  # rows each core owns per group (256)

_CACHE = {}


def _build_program():
    nc = bacc.Bacc(
        "TRN2", target_bir_lowering=False, debug=False, num_devices=N_CORES
    )

    x_d = nc.dram_tensor("x", [T, C], F32, kind="ExternalInput").ap()
    cos_d = nc.dram_tensor("cos", [T, D], F32, kind="ExternalInput").ap()
    sin_d = nc.dram_tensor("sin", [T, D], F32, kind="ExternalInput").ap()
    wq_d = nc.dram_tensor("wq", [C, R * D], F32, kind="ExternalInput").ap()
    wk_d = nc.dram_tensor("wk", [C, D], F32, kind="ExternalInput").ap()
    wv_d = nc.dram_tensor("wv", [C, D], F32, kind="ExternalInput").ap()
    wo_d = nc.dram_tensor("wo", [R * D, C], F32, kind="ExternalInput").ap()
    out_d = nc.dram_tensor("out", [NG * NROW, C], F32, kind="ExternalOutput").ap()

    with tile.TileContext(nc) as tc:
        _kernel_body(tc, x_d, cos_d, sin_d, wq_d, wk_d, wv_d, wo_d, out_d)

    nc.compile()
    return nc


def _kernel_body(tc, x_d, cos_d, sin_d, wq_d, wk_d, wv_d, wo_d, out_d):
    nc = tc.nc

    consts = tc.alloc_tile_pool(name="consts", bufs=1)
    projout = tc.alloc_tile_pool(name="projout", bufs=1)
    wo_pool = tc.alloc_tile_pool(name="wo", bufs=1)
    dram = tc.alloc_tile_pool(name="dram", bufs=1, space="DRAM")

    # --- constants: causal mask (ST layout: keep tk<=tq), cos/sin, identity ---
    ut_mask = consts.tile([128, 128], BF16)
    nc.gpsimd.memset(ut_mask, 1.0)
    nc.gpsimd.affine_select(
        out=ut_mask,
        in_=ut_mask,
        compare_op=mybir.AluOpType.is_ge,
        fill=0.0,
        base=0,
        pattern=[[1, 128]],
        channel_multiplier=-1,
    )

    cos_sb = consts.tile([128, TI, D], F32)
    sin_sb = consts.tile([128, TI, D], F32)
    nc.gpsimd.dma_start(cos_sb, cos_d.rearrange("(to ti) d -> ti to d", ti=128))
    nc.gpsimd.dma_start(sin_sb, sin_d.rearrange("(to ti) d -> ti to d", ti=128))

    ident_f = consts.tile([128, 128], F32)
    make_identity(nc, ident_f)
    ident_b = consts.tile([128, 128], BF16)
    make_identity(nc, ident_b)

    # --- persistent proj outputs, split by token group for fine-grained deps ---
    qt_g = [projout.tile([128, R, GT], BF16, name=f"qt{g}") for g in range(NG)]  # [d, h, tok]
    kt_g = [projout.tile([128, GT], BF16, name=f"kt{g}") for g in range(NG)]  # [d, tok]
    v_g = [projout.tile([128, GB, 132], BF16, name=f"v{g}") for g in range(NG)]  # [tok, kb, d|1]
    yt_g = [projout.tile([128, R, GT], BF16, name=f"yt{g}") for g in range(NG)]  # [d, h, tok]
    for g in range(NG):
        nc.vector.memset(v_g[g][:, :, 128], 1.0)

    wo_bf = wo_pool.tile([128, R, C], BF16)  # [d, h, embd_out]

    # --- load weights (cast to bf16); x loads go first on the sync queue ---
    with tc.tile_pool(name="wts", bufs=1) as wts, tc.tile_pool(
        name="wstage", bufs=3
    ) as wstage:
        wq_bf = wts.tile([128, EO, R * D], BF16)
        wk_bf = wts.tile([128, EO, D], BF16)
        wv_bf = wts.tile([128, EO, D], BF16)
        for eo in range(EO):
            st_q = wstage.tile([128, R * D], F32, tag="wst")
            nc.gpsimd.dma_start(st_q, wq_d[eo * 128 : (eo + 1) * 128, :])
            nc.vector.tensor_copy(wq_bf[:, eo, :], st_q)
            st_k = wstage.tile([128, D], F32, tag="wst_kv")
            nc.gpsimd.dma_start(st_k, wk_d[eo * 128 : (eo + 1) * 128, :])
            nc.vector.tensor_copy(wk_bf[:, eo, :], st_k)
            st_v = wstage.tile([128, D], F32, tag="wst_kv")
            nc.gpsimd.dma_start(st_v, wv_d[eo * 128 : (eo + 1) * 128, :])
            nc.vector.tensor_copy(wv_bf[:, eo, :], st_v)
        for h in range(R):
            for no in range(4):
                st_o = wstage.tile([128, 512], F32, tag="wst")
                nc.gpsimd.dma_start(
                    st_o, wo_d[h * 128 : (h + 1) * 128, no * 512 : (no + 1) * 512]
                )
                nc.scalar.copy(wo_bf[:, h, no * 512 : (no + 1) * 512], st_o)

        # --- per token block: load x, PE-transpose, project Q/K/V, RoPE ---
        with tc.tile_pool(name="xstage", bufs=3) as xstage, tc.tile_pool(
            name="xtb", bufs=3
        ) as xtb_pool, tc.tile_pool(
            name="ppsum", bufs=2, space="PSUM"
        ) as ppsum, tc.tile_pool(
            name="kvpsum", bufs=1, space="PSUM"
        ) as kvpsum, tc.tile_pool(
            name="tpsum", bufs=2, space="PSUM"
        ) as tpsum, tc.tile_pool(
            name="qtpsum", bufs=2, space="PSUM"
        ) as qtpsum, tc.tile_pool(name="rope", bufs=2) as rope:
            for ti in range(TI):
                g = ti // GB
                tl = ti * 128 - g * GT  # local token offset within group
                qt_bf, kt_bf, v_bf = qt_g[g], kt_g[g], v_g[g]

                xf = xstage.tile([128, C], F32, tag="xf")
                nc.sync.dma_start(xf, x_d[ti * 128 : (ti + 1) * 128, :])
                xb = xstage.tile([128, C], BF16, tag="xb")
                nc.scalar.copy(xb, xf)
                xt_blk = xtb_pool.tile([128, EO, 128], BF16, tag="xt")
                for e4 in range(4):
                    tp = tpsum.tile([128, 4, 128], BF16, tag="tp")
                    for k in range(4):
                        eo = e4 * 4 + k
                        nc.tensor.transpose(
                            tp[:, k, :], xb[:, eo * 128 : (eo + 1) * 128], ident_b
                        )
                    nc.vector.tensor_copy(xt_blk[:, e4 * 4 : e4 * 4 + 4, :], tp)

                # Q: psum [tok, R*D]
                psq = ppsum.tile([128, R * D], F32, tag="psq")
                for eo in range(EO):
                    nc.tensor.matmul(
                        psq,
                        lhsT=xt_blk[:, eo, :],
                        rhs=wq_bf[:, eo, :],
                        start=(eo == 0),
                        stop=(eo == EO - 1),
                    )
                psq_v = psq[:, :].rearrange("p (h d) -> p h d", h=R)
                cos_bc = cos_sb[:, ti, None, :].to_broadcast((128, R, D))
                sin_bc = sin_sb[:, ti, None, :].to_broadcast((128, R, D))
                tc_t = rope.tile([128, R, D], F32, tag="ropeC")
                ts_t = rope.tile([128, R, D], F32, tag="ropeS")
                nc.vector.tensor_tensor(tc_t, psq_v, cos_bc, MULT)
                nc.vector.tensor_tensor(ts_t, psq_v, sin_bc, MULT)
                qb = rope.tile([128, R, D], BF16, tag="qb")
                nc.vector.tensor_sub(
                    qb[:, :, 0:64], tc_t[:, :, 0:64], ts_t[:, :, 64:128]
                )
                nc.vector.tensor_add(
                    qb[:, :, 64:128], tc_t[:, :, 64:128], ts_t[:, :, 0:64]
                )
                qtp = qtpsum.tile([128, R, 128], BF16, tag="qtp")
                for h in range(R):
                    nc.tensor.transpose(qtp[:, h, :], qb[:, h, :], ident_b)
                nc.vector.tensor_copy(qt_bf[:, :, tl : tl + 128], qtp)

                # K: psum [tok, D]
                psk = kvpsum.tile([128, D], F32, tag="pskv")
                for eo in range(EO):
                    nc.tensor.matmul(
                        psk,
                        lhsT=xt_blk[:, eo, :],
                        rhs=wk_bf[:, eo, :],
                        start=(eo == 0),
                        stop=(eo == EO - 1),
                    )
                tck = rope.tile([128, D], F32, tag="ropeCk")
                tsk = rope.tile([128, D], F32, tag="ropeSk")
                nc.vector.tensor_tensor(tck, psk, cos_sb[:, ti, :], MULT)
                nc.vector.tensor_tensor(tsk, psk, sin_sb[:, ti, :], MULT)
                kb_t = rope.tile([128, D], BF16, tag="kb")
                nc.vector.tensor_sub(kb_t[:, 0:64], tck[:, 0:64], tsk[:, 64:128])
                nc.vector.tensor_add(kb_t[:, 64:128], tck[:, 64:128], tsk[:, 0:64])
                tpk = qtpsum.tile([128, R, 128], BF16, tag="qtp")
                nc.tensor.transpose(tpk[:, 0, :], kb_t, ident_b)
                nc.vector.tensor_copy(kt_bf[:, tl : tl + 128], tpk[:, 0, :])

                # V: psum [tok, D] -> v_bf[:, kb_local, 0:128]; col 128 = 1.0
                psv = kvpsum.tile([128, D], F32, tag="pskv")
                for eo in range(EO):
                    nc.tensor.matmul(
                        psv,
                        lhsT=xt_blk[:, eo, :],
                        rhs=wv_bf[:, eo, :],
                        start=(eo == 0),
                        stop=(eo == EO - 1),
                    )
                nc.scalar.copy(v_bf[:, ti - g * GB, 0:128], psv)

    # --- attention + Wo + ReduceScatter, pipelined over token groups ---
    groups = [[0, 1, 2, 3], [4, 5, 6, 7]]
    with tc.tile_pool(name="st", bufs=2) as stp, tc.tile_pool(
        name="apsum", bufs=2, space="PSUM"
    ) as apsum, tc.tile_pool(
        name="ytpsum", bufs=1, space="PSUM"
    ) as ytpsum, tc.tile_pool(
        name="wopsum", bufs=1, space="PSUM"
    ) as wopsum, tc.tile_pool(name="ypool", bufs=3) as ypool, tc.tile_pool(
        name="outp", bufs=2
    ) as outp:
        st_max = max(
            sum((g + 1) * GT - max(kb * 128, g * GT) for kb in range((g + 1) * GB))
            for g in range(NG)
        )
        for g in range(NG):
            lo, hi = g * GT, (g + 1) * GT
            offs = {}
            o = 0
            for kb in range((g + 1) * GB):
                offs[kb] = o
                o += hi - max(kb * 128, lo)

            for h in range(R):
                st_all = stp.tile([128, st_max], BF16, tag="st_all")
                for kb in range((g + 1) * GB):
                    s0 = max(kb * 128, lo)
                    w = hi - s0
                    ps = apsum.tile([128, 1024], F32, tag="strip")
                    for m0 in range(0, w, 512):
                        mw = min(512, w - m0)
                        nc.tensor.matmul(
                            ps[:, m0 : m0 + mw],
                            lhsT=kt_g[kb // GB][
                                :, kb * 128 - (kb // GB) * GT : (kb + 1) * 128 - (kb // GB) * GT
                            ],
                            rhs=qt_g[g][:, h, s0 - lo + m0 : s0 - lo + m0 + mw],
                            start=True,
                            stop=True,
                        )
                    nc.scalar.activation(
                        st_all[:, offs[kb] : offs[kb] + w],
                        ps[:, :w],
                        EXP,
                        scale=SCALE,
                    )
                    if kb * 128 >= lo:  # diagonal block lives in this group
                        nc.vector.tensor_mul(
                            st_all[:, offs[kb] : offs[kb] + 128],
                            st_all[:, offs[kb] : offs[kb] + 128],
                            ut_mask,
                        )
                # AV: per query block j in this group, accumulate over kb<=j
                for j in range(g * GB, (g + 1) * GB):
                    po = apsum.tile([128, 132], F32, tag="po")
                    for kb in range(j + 1):
                        s = offs[kb] + j * 128 - max(kb * 128, lo)
                        nc.tensor.matmul(
                            po[:, 0:129],
                            lhsT=st_all[:, s : s + 128],
                            rhs=v_g[kb // GB][:, kb - (kb // GB) * GB, 0:129],
                            start=(kb == 0),
                            stop=(kb == j),
                        )
                    rec = ypool.tile([128, 1], F32, tag="rec")
                    nc.vector.reciprocal(rec, po[:, 128:129])
                    yb = ypool.tile([128, 128], BF16, tag="yb")
                    nc.vector.tensor_scalar_mul(yb, po[:, 0:128], rec)
                    ytp = ytpsum.tile([128, 128], BF16, tag="ytp")
                    nc.tensor.transpose(ytp, yb, ident_b)
                    jl = j * 128 - lo
                    nc.vector.tensor_copy(yt_g[g][:, h, jl : jl + 128], ytp)

            # Wo partial rows for this group (f32 copy + single bf16 cast)
            partial_g = dram.tile([GT, C], BF16, tag=f"partial{g}")
            for tb in range(GB):
                osb = outp.tile([128, C], BF16, tag="osb")
                for no in range(4):
                    pw = wopsum.tile([128, 512], F32, tag="pw")
                    for hh in range(R):
                        nc.tensor.matmul(
                            pw,
                            lhsT=yt_g[g][:, hh, tb * 128 : (tb + 1) * 128],
                            rhs=wo_bf[:, hh, no * 512 : (no + 1) * 512],
                            start=(hh == 0),
                            stop=(hh == R - 1),
                        )
                    if no == 3:
                        nc.scalar.copy(osb[:, no * 512 : (no + 1) * 512], pw)
                    else:
                        nc.vector.tensor_copy(osb[:, no * 512 : (no + 1) * 512], pw)
                nc.sync.dma_start(partial_g[tb * 128 : (tb + 1) * 128, :], osb)

            rs_g = dram.tile([NROW, C], BF16, tag=f"rs{g}")
            nc.gpsimd.collective_compute(
                "ReduceScatter",
                mybir.AluOpType.add,
                replica_groups=groups,
                ins=[partial_g.opt()],
                outs=[rs_g.opt()],
            )
            # cast my 256-row share to f32 and store to local output rows
            for blk in range(NROW // 128):
                rsb = outp.tile([128, C], BF16, tag="rsb")
                nc.sync.dma_start(rsb, rs_g[blk * 128 : (blk + 1) * 128, :])
                osf2 = outp.tile([128, C], F32, tag="osf2")
                nc.vector.tensor_copy(osf2[:, 0:1024], rsb[:, 0:1024])
                nc.scalar.copy(osf2[:, 1024:2048], rsb[:, 1024:2048])
                nc.sync.dma_start(
                    out_d[g * NROW + blk * 128 : g * NROW + (blk + 1) * 128, :],
                    osf2,
                )

    for pool in (dram, wo_pool, projout, consts):
        pool.release()


def _shard_inputs(x, cos, sin, Wq, Wkv, Wo):
    in_maps = []
    for c in range(N_CORES):
        b, g = c // KH, c % KH
        in_maps.append(
            {
                "x": np.ascontiguousarray(x[b], dtype=np.float32),
                "cos": np.ascontiguousarray(cos, dtype=np.float32),
                "sin": np.ascontiguousarray(sin, dtype=np.float32),
                "wq": np.ascontiguousarray(
                    Wq[:, g * R * D : (g + 1) * R * D], dtype=np.float32
                ),
                "wk": np.ascontiguousarray(
                    Wkv[:, g * D : (g + 1) * D], dtype=np.float32
                ),
                "wv": np.ascontiguousarray(
                    Wkv[:, KH * D + g * D : KH * D + (g + 1) * D], dtype=np.float32
                ),
                "wo": np.ascontiguousarray(
                    Wo[g * R * D : (g + 1) * R * D, :], dtype=np.float32
                ),
            }
        )
    return in_maps


def get_program():
    if "nc" not in _CACHE:
        _CACHE["nc"] = _build_program()
    return _CACHE["nc"]


def run(x, cos, sin, Wq, Wkv, Wo, **spmd_kwargs):
    nc = get_program()
    in_maps = _shard_inputs(x, cos, sin, Wq, Wkv, Wo)
    res = run_bass_kernel_spmd(
        nc, in_maps, core_ids=list(range(N_CORES)), **spmd_kwargs
    )
    # core (b, r) holds rows [g*GT + r*NROW, +NROW) of batch b at local
    # offset [g*NROW, +NROW) for each token group g.
    out = np.empty((B, T, C), dtype=np.float32)
    for b in range(B):
        for r in range(KH):
            loc = res.results[b * KH + r]["out"]
            for g in range(NG):
                out[b, g * GT + r * NROW : g * GT + (r + 1) * NROW] = loc[
                    g * NROW : (g + 1) * NROW
                ]
    return out, res


def kernel(x, cos, sin, Wq, Wkv, Wo):
    out, _ = run(x, cos, sin, Wq, Wkv, Wo)
    return out
